# revision 7
# baseline (speedup 1.0000x reference)
"""Trainium2 Bass kernel for nn_Custom_Pooling_3D.

Math (from the reference): the 0/1 matrix T encodes a fixed 2x2 spatial
sum-pool over a [I=32, J=32, C=16] layout (basis index i*512 + j*16 + c),
producing [O=16, O=16, C=16] (index oi*256 + oj*16 + c):

    y[b, oi, oj, c] = sqrt( sum_{di,dj in {0,1}} x[b, 2oi+di, 2oj+dj, c]^2 )

So T is never needed on device; the pooling structure is hardcoded.

Sharding: data-parallel over batch. 1024 rows / 8 cores = 128 rows per
core = exactly the 128 SBUF partitions.

The kernel is DMA-bound, so all device I/O is fp16 (the host converts
f32 -> fp16 on the way in and fp16 -> f32 on the way out; the rel-err
budget of 2e-2 dwarfs fp16's ~5e-4 rounding).  Per core that is
4 MiB of loads + 0.5 MiB of stores at ~360 GB/s -> ~13.1 us of DMA
occupancy.  Compute hides under that: squares are split between ACT
(activation Square) and DVE (tensor_mul) so neither engine exceeds the
DMA budget; the two pooling adds run on DVE in fp16 (2x_1p mode, 2-byte
packed operands -> half cycles); sqrt runs on ACT.

Per-chunk pipeline: load -> square -> j-pair add -> i-pair add -> sqrt
-> store, streamed over tapered column chunks (big first while the pipe
fills, small last to shrink the serial drain tail).  Early stores
dispatch from the Pool/SWDGE sequencer so their sqrt-waits cannot
head-of-line-block load dispatches on SP; the last stores go back on SP
(drained by then) for the lower-latency HWDGE path.
"""

import os
import sys

import numpy as np

for _p in ("/opt/trn_rl_repo", "/root/.axon_site/_ro/trn_rl_repo"):
    if os.path.isdir(_p) and _p not in sys.path:
        sys.path.insert(0, _p)

import concourse.tile as tile
from concourse import bacc, mybir
from concourse.bass_utils import run_bass_kernel_spmd

N_CORES = 8
BATCH = 1024
IN_F = 16384  # 32 * 32 * 16  (i, j, c)
OUT_F = 4096  # 16 * 16 * 16  (oi, oj, c)
BSH = BATCH // N_CORES  # 128 rows per core == SBUF partition count

# Input-column widths per chunk (each a multiple of 1024 so every chunk
# holds whole oi-pairs).  Small first so compute starts early, big in the
# middle while the pipe is full, small last to shrink the drain tail.
CHUNKS = [1024, 2048, 2048, 2048, 2048, 2048, 2048, 2048, 1024]
# Each chunk's square is column-split across up to three engines, all
# writing into one zt tile: DVE (0.52 ns/elem fp16 2x_1p, also owns the
# two pooling adds), ACT (0.833 ns/elem, also owns the sqrts), and Pool
# (1.98 ns/elem, otherwise idle).  The split is sized so every engine's
# per-chunk work fits inside the chunk's own DMA load time, keeping all
# engines in steady-state cadence behind the loads instead of letting
# one engine's queue become the critical chain.
SQ_SPLITS = [
    (1024, 0, 0),
    (1024, 512, 512),
    (1024, 512, 512),
    (1024, 512, 512),
    (1024, 512, 512),
    (1024, 512, 512),
    (1024, 512, 512),
    (1024, 512, 512),
    (1024, 0, 0),
]
# Stores ride HWDGE off sequencers that are idle by then (SP after the
# loads, ACT's SEQ otherwise); Pool is kept clear for its square slices.
STORE_ENGS = ["sync", "scalar"] * 4 + ["sync"]

_CACHE = {}


def _build_program(chunks=None, bufs=None, store_engs=None, sq_splits=None):
    chunks = chunks or CHUNKS
    assert sum(chunks) == IN_F and all(c % 1024 == 0 for c in chunks)
    n = len(chunks)
    # One buffer per chunk in every pool: total SBUF is only ~96 KiB per
    # partition in fp16, and distinct buffers mean loads never wait on
    # slot-reuse (WAR) behind compute.
    bufs = bufs or dict(xp=n, zp=n, tp=n, rp=n, op=n)
    if sq_splits is None:
        sq_splits = SQ_SPLITS if n == len(SQ_SPLITS) else [
            (c, 0, 0) for c in chunks
        ]
    assert all(sum(s) == c for s, c in zip(sq_splits, chunks))
    if store_engs is None:
        store_engs = STORE_ENGS if n == len(STORE_ENGS) else (
            ["sync", "scalar"] * (n // 2) + ["sync"] * (n % 2)
        )

    # Bacc (not plain Bass): its compile() runs generate_event_semaphores,
    # which legalizes to TRN2's 1-wait-per-instruction limit.
    nc = bacc.Bacc("TRN2", target_bir_lowering=False, debug=False)
    f16 = mybir.dt.float16
    AF = mybir.ActivationFunctionType
    x = nc.dram_tensor("x", [BSH, IN_F], f16, kind="ExternalInput").ap()
    y = nc.dram_tensor("y", [BSH, OUT_F], f16, kind="ExternalOutput").ap()

    xoffs = [sum(chunks[:k]) for k in range(n)]
    yoffs = [xo // 4 for xo in xoffs]

    with tile.TileContext(nc) as tc:
        with (
            tc.tile_pool(name="xp", bufs=bufs["xp"]) as xp,
            tc.tile_pool(name="zp", bufs=bufs["zp"]) as zp,
            tc.tile_pool(name="tp", bufs=bufs["tp"]) as tp,
            tc.tile_pool(name="rp", bufs=bufs["rp"]) as rp,
            tc.tile_pool(name="op", bufs=bufs["op"]) as op,
        ):
            for k in range(n):
                cin = chunks[k]
                cout = cin // 4
                ni = cin // 512
                # load
                xt = xp.tile([BSH, cin], f16, tag="xt")
                nc.sync.dma_start(xt[:, :], x[:, xoffs[k] : xoffs[k] + cin])
                # square, column-split across DVE / ACT / Pool into one tile
                zt = zp.tile([BSH, cin], f16, tag="zt")
                cd, ca, cp = sq_splits[k]
                if cd:
                    nc.vector.tensor_mul(zt[:, :cd], xt[:, :cd], xt[:, :cd])
                if ca:
                    nc.scalar.activation(
                        zt[:, cd : cd + ca], xt[:, cd : cd + ca], AF.Square
                    )
                if cp:
                    nc.gpsimd.tensor_mul(
                        zt[:, cd + ca :], xt[:, cd + ca :], xt[:, cd + ca :]
                    )
                # j-pair add: [i, oj(16), 2, c(16)] -> [i, oj(16), c(16)]
                z = zt[:, :].rearrange(
                    "p (i oj two c) -> p i oj two c", i=ni, oj=16, two=2, c=16
                )
                tt = tp.tile([BSH, 2 * cout], f16, tag="tt")
                t4 = tt[:, :].rearrange(
                    "p (i oj c) -> p i oj c", i=ni, oj=16, c=16
                )
                nc.vector.tensor_add(t4, z[:, :, :, 0, :], z[:, :, :, 1, :])
                # i-pair add: [oi, 2, m(256)] -> [oi, m(256)]
                t3 = tt[:, :].rearrange(
                    "p (oi two m) -> p oi two m", oi=ni // 2, two=2, m=256
                )
                rt = rp.tile([BSH, cout], f16, tag="rt")
                r3 = rt[:, :].rearrange("p (oi m) -> p oi m", oi=ni // 2, m=256)
                nc.vector.tensor_add(r3, t3[:, :, 0, :], t3[:, :, 1, :])
                # sqrt to its own tile, then store (engine per store_engs)
                ot = op.tile([BSH, cout], f16, tag="ot")
                nc.scalar.activation(ot[:, :], rt[:, :], AF.Sqrt)
                getattr(nc, store_engs[k]).dma_start(
                    y[:, yoffs[k] : yoffs[k] + cout], ot[:, :]
                )
    nc.compile()
    _fuse_act_table_loads(nc, {AF.Square, AF.Sqrt})
    return nc


def _fuse_act_table_loads(nc, funcs_used):
    """bacc's insert_act_table_loads picks the first table set per function,
    which here yields two loads (square -> set 0, sqrt -> set 3) at ~2.7us
    each.  One set (sqrt_and_others) contains both; patch the first load to
    it and drop the rest.  Loads carry no sync info, so deletion is safe."""
    from concourse.hw_specs import get_activation_tables

    tabs = list(get_activation_tables(nc.m.arch).items())
    combined = next(
        (i for i, (_, fns) in enumerate(tabs) if funcs_used <= fns), None
    )
    if combined is None:
        return
    for blk in nc.m.functions[0].blocks:
        insts = blk.instructions  # live list view
        loads = [i for i in insts if type(i).__name__ == "InstLoadActFuncSet"]
        if len(loads) <= 1:
            continue
        if any(i.sync_info and (i.sync_info.on_wait or i.sync_info.on_update)
               for i in loads):
            continue
        loads[0].act_func_set_id = combined
        for extra in loads[1:]:
            insts.remove(extra)


def _run(x_full, trace=False, tmpdir=None):
    """x_full: [1024, 16384] f32. Returns (y_full [1024, 4096] f32, results obj)."""
    if "nc" not in _CACHE:
        _CACHE["nc"] = _build_program()
    nc = _CACHE["nc"]
    x16 = np.ascontiguousarray(x_full.astype(np.float16))
    in_maps = [
        {"x": x16[c * BSH : (c + 1) * BSH]} for c in range(N_CORES)
    ]
    res = run_bass_kernel_spmd(
        nc, in_maps, list(range(N_CORES)), trace=trace, tmpdir=tmpdir
    )
    y_full = np.concatenate(
        [res.results[c]["y"] for c in range(N_CORES)], axis=0
    ).astype(np.float32)
    return y_full, res


def kernel(input_state, T=None, **_unused):
    x = np.asarray(input_state, dtype=np.float32)
    assert x.shape == (BATCH, IN_F), x.shape
    y, _ = _run(x, trace=False)
    return y
